# revision 15
# baseline (speedup 1.0000x reference)
"""Multi-Head Latent Attention (MLA) prefill kernel for 8 Trainium2 NeuronCores.

Sharding: latent down-projections row-split 8 ways + AllGather; up-projections
and attention head-split (2 heads/core); AllToAll converts head-split attention
output to token-split for the output projection. Host only slices inputs and
concatenates the per-core output row slabs.
"""
import sys
if '/opt/trn_rl_repo' not in sys.path:
    sys.path.insert(0, '/opt/trn_rl_repo')

import math
import numpy as np

import concourse.bass as bass
import concourse.tile as tile
import concourse.mybir as mybir
from concourse import bacc

F32 = mybir.dt.float32
F32R = mybir.dt.float32r
AF = mybir.ActivationFunctionType
ALU = mybir.AluOpType

B, S, DIM, H = 2, 2048, 2048, 16
NOPE, ROPE, QKD, VD = 128, 64, 192, 128
QLR, KVLR = 512, 512
EPS = 1e-6
NC = 8
N = B * S              # 4096 flattened tokens
R = N // NC            # 512 tokens per core (phase 1 / output rows)
HPC = H // NC          # 2 heads per core
NBLK = N // R          # 8 token blocks (= AG shards)
SCALE = 1.0 / math.sqrt(QKD)

SKIP, PLAIN = -2, -1   # mask block classes (>=0 -> index into mask blocks)


def _rope_tables():
    freqs = (1.0 / (10000.0 ** (np.arange(0, ROPE, 2, dtype=np.float32) / ROPE)))
    ang = np.arange(S, dtype=np.float32)[:, None] * freqs[None, :]      # [S, 32]
    return np.cos(ang).T.copy(), np.sin(ang).T.copy()                   # [32, S]


def _classify_mask(mask):
    """Per (q-chunk of 512, k-block of 128): SKIP / PLAIN / index of mask block.

    Returns (cls[4][16], blocks [nblk,128,512] transposed mask as f32)."""
    cls = [[PLAIN] * (S // 128) for _ in range(S // 512)]
    blocks = []
    for qc in range(S // 512):
        sub_q = mask[qc * 512:(qc + 1) * 512]
        for kb in range(S // 128):
            blk = sub_q[:, kb * 128:(kb + 1) * 128]
            if not blk.any():
                cls[qc][kb] = SKIP
            elif blk.all():
                cls[qc][kb] = PLAIN
            else:
                cls[qc][kb] = len(blocks)
                blocks.append(blk.T.astype(np.float32))   # [128 k, 512 q]
    blocks = (np.stack(blocks) if blocks
              else np.zeros((0, 128, 512), np.float32))
    return cls, blocks


def _build(cls, nmask, flags):
    """Emit the bass program. cls/nmask/flags are compile-time schedule data."""
    nc = bacc.Bacc(None, num_devices=NC)

    # ---- I/O ----
    x_c = nc.dram_tensor("x_c", [R, DIM], F32, kind="ExternalInput")
    wqaT = nc.dram_tensor("wqaT", [DIM, QLR], F32, kind="ExternalInput")
    bqa = nc.dram_tensor("bqa", [QLR], F32, kind="ExternalInput")
    wkvaT = nc.dram_tensor("wkvaT", [DIM, KVLR + ROPE], F32, kind="ExternalInput")
    bkva = nc.dram_tensor("bkva", [KVLR + ROPE], F32, kind="ExternalInput")
    qnw = nc.dram_tensor("qnw", [QLR], F32, kind="ExternalInput")
    kvnw = nc.dram_tensor("kvnw", [KVLR], F32, kind="ExternalInput")
    # trig: phase-1 (this core's 512 positions) & phase-2 (full 2048 positions)
    trig1c = nc.dram_tensor("trig1c", [64, R], F32, kind="ExternalInput")   # [cos;sin]
    trig2c = nc.dram_tensor("trig2c", [64, R], F32, kind="ExternalInput")   # [sin;cos]
    trigq1 = nc.dram_tensor("trigq1", [128, S], F32, kind="ExternalInput")  # [c;s;c;s]
    trigq2 = nc.dram_tensor("trigq2", [128, S], F32, kind="ExternalInput")  # [s;c;s;c]
    sgn = nc.dram_tensor("sgn", [128, 1], F32, kind="ExternalInput")        # rope comb sign
    wqbT = nc.dram_tensor("wqbT", [QLR, HPC * QKD], F32, kind="ExternalInput")
    bqb = nc.dram_tensor("bqb", [HPC * QKD], F32, kind="ExternalInput")
    wkbT = nc.dram_tensor("wkbT", [KVLR, HPC * NOPE], F32, kind="ExternalInput")
    bkb = nc.dram_tensor("bkb", [HPC * NOPE], F32, kind="ExternalInput")
    wvbT = nc.dram_tensor("wvbT", [KVLR, HPC * VD], F32, kind="ExternalInput")
    bvb = nc.dram_tensor("bvb", [1, HPC * VD], F32, kind="ExternalInput")
    woT = nc.dram_tensor("woT", [H * VD, DIM], F32, kind="ExternalInput")
    wob = nc.dram_tensor("wob", [1, DIM], F32, kind="ExternalInput")
    maskblk = nc.dram_tensor("maskblk", [max(nmask, 1), 128, 512], F32,
                             kind="ExternalInput")
    out_c = nc.dram_tensor("out", [R, DIM], F32, kind="ExternalOutput")

    LAT = KVLR + ROPE  # 576

    with tile.TileContext(nc) as tc:
        with tc.tile_pool(name="konst", bufs=1) as konst, \
             tc.tile_pool(name="dram", bufs=1, space="DRAM") as dram:
            qkv = tc.alloc_tile_pool(name="qkv", bufs=1)

            # ---- constants ----
            ones_f = konst.tile([128, 1], F32)
            nc.vector.memset(ones_f[:, :], 1.0)
            ones_col = konst.tile([128, 1], F32R)
            nc.vector.tensor_copy(out=ones_col[:, :], in_=ones_f[:, :])
            ones_rf = konst.tile([1, 128], F32)
            nc.vector.memset(ones_rf[:, :], 1.0)
            ones_row = konst.tile([1, 128], F32R)
            nc.vector.tensor_copy(out=ones_row[:, :], in_=ones_rf[:, :])
            sgn_t = konst.tile([128, 1], F32)
            nc.sync.dma_start(out=sgn_t, in_=sgn[:, :])
            eps_t = konst.tile([1, 1], F32)
            nc.vector.memset(eps_t[:, :], EPS)

            # ---- collective DRAM tiles ----
            ag_in = dram.tile([QLR + LAT, R], F32)                 # [1088, 512]
            ag_out = dram.tile([NC, QLR + LAT, R], F32, addr_space="Shared")
            a2a_in = dram.tile([NC, HPC * VD, R], F32)             # [8, 256, 512]
            a2a_out = dram.tile([NC, HPC * VD, R], F32)

            # ---- persistent q/k/v (phase 2 -> phase 3) ----
            q_nope = [qkv.tile([128, N], F32R, name=f"q_nope{i}") for i in range(HPC)]
            q_ropeP = qkv.tile([128, N], F32R)   # [h0r1;h0r2;h1r1;h1r2] x 32 rows
            k_nope = [qkv.tile([128, N], F32R, name=f"k_nope{i}") for i in range(HPC)]
            k_pe2 = qkv.tile([128, N], F32R)     # rows 0-63 and 64-127 identical
            vt = qkv.tile([128, N // 128, HPC * VD], F32R)   # token-major V

            # ================= PHASE 1: latent down-proj (row shard) ============
            with tc.tile_pool(name="p1sb", bufs=1) as p1sb, \
                 tc.tile_pool(name="p1x", bufs=1) as p1x, \
                 tc.tile_pool(name="p1w", bufs=4) as p1w, \
                 tc.tile_pool(name="p1tmp", bufs=1) as p1tmp, \
                 tc.tile_pool(name="p1ps", bufs=3, space="PSUM") as p1ps, \
                 tc.tile_pool(name="p1ps2", bufs=2, space="PSUM") as p1ps2, \
                 tc.tile_pool(name="p1ps1", bufs=1, space="PSUM") as p1ps1:

                t1c = p1sb.tile([64, R], F32)
                nc.sync.dma_start(out=t1c, in_=trig1c[:, :])
                t2c = p1sb.tile([64, R], F32)
                nc.sync.dma_start(out=t2c, in_=trig2c[:, :])

                xts = [p1x.tile([128, R], F32R, tag=f"xt{k}", name=f"xt{k}")
                       for k in range(DIM // 128)]
                for k in range(DIM // 128):
                    nc.sync.dma_start(
                        out=xts[k],
                        in_=x_c[:, k * 128:(k + 1) * 128]
                        .rearrange("t d -> d t").bitcast(F32R))

                for path in range(2):  # 0: q, 1: kv
                    wT, bias_d, normw_d = ((wqaT, bqa, qnw) if path == 0
                                           else (wkvaT, bkva, kvnw))
                    nm = 4 if path == 0 else 5  # kv has extra 64-row rope chunk
                    acts = []
                    sums_ps = p1ps1.tile([1, R], F32, name=f"sums{path}",
                                         tag="sums")
                    for m in range(nm):
                        mp = 64 if m == 4 else 128
                        ps = p1ps.tile([128, R], F32, tag="p1acc")
                        for k in range(DIM // 128):
                            wt = p1w.tile([128, mp], F32R, tag="w1")
                            nc.sync.dma_start(
                                out=wt[:, :],
                                in_=wT[k * 128:(k + 1) * 128,
                                       m * 128:m * 128 + mp].bitcast(F32R))
                            nc.tensor.matmul(ps[0:mp, :], wt[:, 0:mp], xts[k][:, :],
                                             start=(k == 0), stop=(k == DIM // 128 - 1))
                        a = p1sb.tile([128, R], F32, tag=f"act{m}",
                                      name=f"a{path}{m}")
                        if flags['ba'][path]:
                            bt = p1sb.tile([128, 1], F32, tag=f"bias{m}",
                                           name=f"b{path}{m}")
                            nc.sync.dma_start(out=bt[0:mp, :],
                                              in_=bias_d[m * 128:m * 128 + mp]
                                              .rearrange("(a b) -> a b", b=1))
                            nc.vector.tensor_scalar_add(a[0:mp, :], ps[0:mp, :],
                                                        bt[0:mp, :])
                        else:
                            nc.scalar.activation(out=a[0:mp, :], in_=ps[0:mp, :],
                                                 func=AF.Copy)
                        acts.append(a)
                        if m < 4:   # latent chunks: accumulate sum of squares
                            sq = p1tmp.tile([128, R], F32, tag="sq")
                            nc.vector.tensor_mul(sq[:, :], a[:, :], a[:, :])
                            nc.tensor.matmul(sums_ps[:, :], ones_f[:, :], sq[:, :],
                                             start=(m == 0), stop=(m == 3),
                                             skip_group_check=True)
                    # rstd = 1/sqrt(mean + eps)
                    std = p1tmp.tile([1, R], F32, tag="std")
                    nc.scalar.activation(out=std[:, :], in_=sums_ps[:, :],
                                         func=AF.Sqrt, scale=1.0 / (QLR if path == 0 else KVLR),
                                         bias=eps_t[:, :])
                    rstd_f = p1tmp.tile([1, R], F32, tag="rstdf")
                    nc.vector.reciprocal(out=rstd_f[:, :], in_=std[:, :])
                    rstd = p1tmp.tile([1, R], F32R, tag="rstd")
                    nc.vector.tensor_copy(out=rstd[:, :], in_=rstd_f[:, :])
                    for m in range(4):
                        wrow = p1sb.tile([1, 128], F32R, tag=f"wrow{m}",
                                         name=f"w{path}{m}")
                        nc.sync.dma_start(out=wrow,
                                          in_=normw_d[m * 128:(m + 1) * 128]
                                          .rearrange("(b a) -> b a", b=1).bitcast(F32R))
                        rep = p1ps2.tile([128, R], F32, tag="p1rep")
                        nc.tensor.matmul(rep[:, :], wrow[:, :], rstd[:, :],
                                         start=True, stop=True)
                        nrm = p1tmp.tile([128, R], F32, tag="nrm")
                        nc.vector.tensor_mul(nrm[:, :], acts[m][:, :], rep[:, :])
                        nc.sync.dma_start(
                            out=ag_in[path * QLR + m * 128:path * QLR + (m + 1) * 128, :],
                            in_=nrm[:, :])
                    if path == 1:   # rope on k_pe chunk [64, R]
                        kpe = acts[4]
                        u = p1tmp.tile([64, R], F32, tag="u1")
                        nc.vector.tensor_mul(u[:, :], kpe[0:64, :], t1c[:, :])
                        w = p1tmp.tile([64, R], F32, tag="w1t")
                        nc.vector.tensor_mul(w[:, :], kpe[0:64, :], t2c[:, :])
                        z1 = p1tmp.tile([64, R], F32, tag="z1")
                        nc.sync.dma_start(out=z1[0:32, :], in_=u[32:64, :])
                        nc.sync.dma_start(out=z1[32:64, :], in_=w[0:32, :])
                        y1 = p1tmp.tile([64, R], F32, tag="y1")
                        nc.sync.dma_start(out=y1[0:32, :], in_=u[0:32, :])
                        nc.sync.dma_start(out=y1[32:64, :], in_=w[32:64, :])
                        rot = p1tmp.tile([64, R], F32, tag="rot1")
                        nc.vector.scalar_tensor_tensor(
                            out=rot[:, :], in0=z1[:, :], scalar=sgn_t[0:64, :],
                            in1=y1[:, :], op0=ALU.mult, op1=ALU.add)
                        nc.sync.dma_start(out=ag_in[2 * QLR:2 * QLR + ROPE, :],
                                          in_=rot[:, :])

            # ---- AllGather ----
            nc.gpsimd.collective_compute(
                "AllGather", ALU.bypass,
                replica_groups=[list(range(NC))],
                ins=[ag_in.opt()], outs=[ag_out.opt()])

            # ================= PHASE 2: per-head up-projections ================
            with tc.tile_pool(name="p2w", bufs=1) as p2w, \
                 tc.tile_pool(name="p2lat", bufs=1) as p2lat, \
                 tc.tile_pool(name="p2tmp", bufs=2) as p2tmp, \
                 tc.tile_pool(name="p2ps", bufs=4, space="PSUM") as p2ps:

                tq1 = p2w.tile([128, S], F32)
                nc.sync.dma_start(out=tq1, in_=trigq1[:, :])
                tq2 = p2w.tile([128, S], F32)
                nc.sync.dma_start(out=tq2, in_=trigq2[:, :])
                # weight tiles (held)
                wqb_t = [[p2w.tile([128, 128], F32R, name=f"wqb{k}{m}")
                          for m in range(3)] for k in range(4)]
                for k in range(4):
                    for m in range(3):
                        nc.sync.dma_start(
                            out=wqb_t[k][m],
                            in_=wqbT[k * 128:(k + 1) * 128,
                                     m * 128:(m + 1) * 128].bitcast(F32R))
                wkb_t = [[p2w.tile([128, 128], F32R, name=f"wkb{k}{m}")
                          for m in range(2)] for k in range(4)]
                for k in range(4):
                    for m in range(2):
                        nc.sync.dma_start(
                            out=wkb_t[k][m],
                            in_=wkbT[k * 128:(k + 1) * 128,
                                     m * 128:(m + 1) * 128].bitcast(F32R))
                wvb_t = [p2w.tile([128, HPC * VD], F32R, name=f"wvb{k}")
                         for k in range(4)]
                for k in range(4):
                    nc.sync.dma_start(out=wvb_t[k],
                                      in_=wvbT[k * 128:(k + 1) * 128, :].bitcast(F32R))
                if flags['bvb']:
                    bvb_t = p2w.tile([1, HPC * VD], F32R)
                    nc.sync.dma_start(out=bvb_t, in_=bvb[:, :].bitcast(F32R))
                if flags['bqb']:
                    bq_t = [p2w.tile([128, 1], F32, name=f"bqt{m}") for m in range(3)]
                    for m in range(3):
                        nc.sync.dma_start(out=bq_t[m],
                                          in_=bqb[m * 128:(m + 1) * 128]
                                          .rearrange("(a b) -> a b", b=1))
                if flags['bkb']:
                    bk_t = [p2w.tile([128, 1], F32, name=f"bkt{m}") for m in range(2)]
                    for m in range(2):
                        nc.sync.dma_start(out=bk_t[m],
                                          in_=bkb[m * 128:(m + 1) * 128]
                                          .rearrange("(a b) -> a b", b=1))

                for s in range(NBLK):
                    tsl = slice(s * R, (s + 1) * R)
                    pos = (s % (S // R)) * R       # position within batch
                    psl = slice(pos, pos + R)
                    qn_t = [p2lat.tile([128, R], F32R, tag=f"qn{k}", name=f"qn{k}")
                            for k in range(4)]
                    kn_t = [p2lat.tile([128, R], F32R, tag=f"kn{k}", name=f"kn{k}")
                            for k in range(4)]
                    for k in range(4):
                        nc.sync.dma_start(out=qn_t[k],
                                          in_=ag_out[s, k * 128:(k + 1) * 128, :]
                                          .bitcast(F32R))
                        nc.sync.dma_start(out=kn_t[k],
                                          in_=ag_out[s, QLR + k * 128:QLR + (k + 1) * 128, :]
                                          .bitcast(F32R))
                    # q_b
                    for m in range(3):
                        ps = p2ps.tile([128, R], F32, tag="p2acc")
                        for k in range(4):
                            nc.tensor.matmul(ps[:, :], wqb_t[k][m][:, :], qn_t[k][:, :],
                                             start=(k == 0), stop=(k == 3))
                        if m < 2:
                            if flags['bqb']:
                                nc.vector.tensor_scalar_add(q_nope[m][:, tsl],
                                                            ps[:, :], bq_t[m][:, :])
                            else:
                                nc.scalar.activation(out=q_nope[m][:, tsl],
                                                     in_=ps[:, :], func=AF.Copy)
                        else:
                            rst = p2tmp.tile([128, R], F32, tag="rst")
                            if flags['bqb']:
                                nc.vector.tensor_scalar_add(rst[:, :], ps[:, :],
                                                            bq_t[m][:, :])
                            else:
                                nc.scalar.activation(out=rst[:, :], in_=ps[:, :],
                                                     func=AF.Copy)
                            u = p2tmp.tile([128, R], F32, tag="u2")
                            nc.vector.tensor_mul(u[:, :], rst[:, :], tq1[:, psl])
                            w = p2tmp.tile([128, R], F32, tag="w2")
                            nc.vector.tensor_mul(w[:, :], rst[:, :], tq2[:, psl])
                            z = p2tmp.tile([128, R], F32, tag="z2")
                            nc.sync.dma_start(out=z[0:32, :], in_=u[32:64, :])
                            nc.sync.dma_start(out=z[32:64, :], in_=w[0:32, :])
                            nc.sync.dma_start(out=z[64:96, :], in_=u[96:128, :])
                            nc.sync.dma_start(out=z[96:128, :], in_=w[64:96, :])
                            y = p2tmp.tile([128, R], F32, tag="y2")
                            nc.sync.dma_start(out=y[0:32, :], in_=u[0:32, :])
                            nc.sync.dma_start(out=y[32:64, :], in_=w[32:64, :])
                            nc.sync.dma_start(out=y[64:96, :], in_=u[64:96, :])
                            nc.sync.dma_start(out=y[96:128, :], in_=w[96:128, :])
                            nc.vector.scalar_tensor_tensor(
                                out=q_ropeP[:, tsl], in0=z[:, :],
                                scalar=sgn_t[:, :], in1=y[:, :],
                                op0=ALU.mult, op1=ALU.add)
                    # k_nope
                    for m in range(2):
                        ps = p2ps.tile([128, R], F32, tag="p2acc")
                        for k in range(4):
                            nc.tensor.matmul(ps[:, :], wkb_t[k][m][:, :], kn_t[k][:, :],
                                             start=(k == 0), stop=(k == 3))
                        if flags['bkb']:
                            nc.vector.tensor_scalar_add(k_nope[m][:, tsl], ps[:, :],
                                                        bk_t[m][:, :])
                        else:
                            nc.scalar.activation(out=k_nope[m][:, tsl], in_=ps[:, :],
                                                 func=AF.Copy)
                    # v (token-major)
                    for mt in range(4):
                        ps = p2ps.tile([128, HPC * VD], F32, tag="p2v")
                        if flags['bvb']:
                            nc.tensor.matmul(ps[:, :], ones_row[:, :], bvb_t[:, :],
                                             start=True, stop=False)
                        for k in range(4):
                            nc.tensor.matmul(
                                ps[:, :],
                                kn_t[k][:, mt * 128:(mt + 1) * 128],
                                wvb_t[k][:, :],
                                start=(k == 0 and not flags['bvb']),
                                stop=(k == 3))
                        nc.scalar.activation(out=vt[:, s * 4 + mt, :], in_=ps[:, :],
                                             func=AF.Copy)
                    # k_pe
                    nc.sync.dma_start(out=k_pe2[0:64, tsl],
                                      in_=ag_out[s, 2 * QLR:2 * QLR + ROPE, :]
                                      .bitcast(F32R))
                nc.sync.dma_start(out=k_pe2[64:128, :], in_=k_pe2[0:64, :])

            # ================= PHASE 3: attention =============================
            with tc.tile_pool(name="p3m", bufs=1) as p3m, \
                 tc.tile_pool(name="p3p", bufs=3) as p3p, \
                 tc.tile_pool(name="p3o", bufs=3) as p3o, \
                 tc.tile_pool(name="p3sc", bufs=2, space="PSUM") as p3sc, \
                 tc.tile_pool(name="p3out", bufs=2, space="PSUM") as p3out, \
                 tc.tile_pool(name="p3rs", bufs=2, space="PSUM") as p3rs, \
                 tc.tile_pool(name="p3rep", bufs=1, space="PSUM") as p3rep:

                mtiles = [p3m.tile([128, 512], F32R, name=f"mt{i}")
                          for i in range(nmask)]
                for i in range(nmask):
                    nc.sync.dma_start(out=mtiles[i],
                                      in_=maskblk[i, :, :].bitcast(F32R))

                for b in range(B):
                    for lh in range(HPC):
                        rb = slice(lh * 64, lh * 64 + 64)   # rope rows for head lh
                        for qc in range(S // 512):
                            qsl = slice(b * S + qc * 512, b * S + (qc + 1) * 512)
                            out_ps = p3out.tile([128, 512], F32, tag="outp")
                            rs_ps = p3rs.tile([1, 512], F32, tag="rsp")
                            kbs = [kb for kb in range(S // 128)
                                   if cls[qc][kb] != SKIP]
                            for i, kb in enumerate(kbs):
                                ksl = slice(b * S + kb * 128, b * S + kb * 128 + 128)
                                sc = p3sc.tile([128, 512], F32, tag="sc")
                                nc.tensor.matmul(sc[:, :], k_nope[lh][:, ksl],
                                                 q_nope[lh][:, qsl],
                                                 start=True, stop=False)
                                nc.tensor.matmul(sc[:, :], k_pe2[rb, ksl],
                                                 q_ropeP[rb, qsl],
                                                 start=False, stop=True)
                                P = p3p.tile([128, 512], F32R, tag="P")
                                nc.scalar.activation(out=P[:, :], in_=sc[:, :],
                                                     func=AF.Exp, scale=SCALE)
                                if cls[qc][kb] >= 0:
                                    nc.vector.tensor_mul(P[:, :], P[:, :],
                                                         mtiles[cls[qc][kb]][:, :])
                                last = (i == len(kbs) - 1)
                                nc.tensor.matmul(
                                    out_ps[:, :],
                                    vt[:, b * 16 + kb, lh * VD:(lh + 1) * VD],
                                    P[:, :], start=(i == 0), stop=last,
                                    skip_group_check=True)
                                nc.tensor.matmul(rs_ps[:, :], ones_col[:, :], P[:, :],
                                                 start=(i == 0), stop=last,
                                                 skip_group_check=True)
                            inv_f = p3o.tile([1, 512], F32, tag="invf")
                            nc.vector.reciprocal(out=inv_f[:, :], in_=rs_ps[:, :])
                            inv = p3o.tile([1, 512], F32R, tag="inv")
                            nc.vector.tensor_copy(out=inv[:, :], in_=inv_f[:, :])
                            rep = p3rep.tile([128, 512], F32, tag="rep")
                            nc.tensor.matmul(rep[:, :], ones_row[:, :], inv[:, :],
                                             start=True, stop=True)
                            rep_sb = p3o.tile([128, 512], F32, tag="repsb")
                            nc.scalar.activation(out=rep_sb[:, :], in_=rep[:, :],
                                                 func=AF.Copy)
                            ao = p3o.tile([128, 512], F32, tag="ao")
                            nc.vector.tensor_mul(ao[:, :], out_ps[:, :], rep_sb[:, :])
                            nc.sync.dma_start(
                                out=a2a_in[b * 4 + qc, lh * VD:(lh + 1) * VD, :],
                                in_=ao[:, :])

            qkv.release()

            # ---- AllToAll ----
            nc.gpsimd.collective_compute(
                "AllToAll", ALU.bypass,
                replica_groups=[list(range(NC))],
                ins=[a2a_in.opt()], outs=[a2a_out.opt()])

            # ================= PHASE 4: output projection =====================
            with tc.tile_pool(name="p4l", bufs=1) as p4l, \
                 tc.tile_pool(name="p4r", bufs=32) as p4r, \
                 tc.tile_pool(name="p4o", bufs=3) as p4o, \
                 tc.tile_pool(name="p4ps", bufs=4, space="PSUM") as p4ps:

                lt = [p4l.tile([128, 512], F32R, name=f"lt{k}") for k in range(16)]
                av = a2a_out[:, :, :].rearrange("c (h p) t -> (c h) p t", h=2)
                for k in range(16):
                    nc.sync.dma_start(out=lt[k], in_=av[k, :, :].bitcast(F32R))
                if flags['wob']:
                    wob_t = p4l.tile([1, DIM], F32R)
                    nc.sync.dma_start(out=wob_t, in_=wob[:, :].bitcast(F32R))
                for n_ in range(4):
                    rh = [p4r.tile([128, 512], F32R, tag="rh", name=f"rh{n_}{k}")
                          for k in range(16)]
                    for k in range(16):
                        nc.sync.dma_start(
                            out=rh[k],
                            in_=woT[k * 128:(k + 1) * 128,
                                    n_ * 512:(n_ + 1) * 512].bitcast(F32R))
                    for m in range(4):
                        ps = p4ps.tile([128, 512], F32, tag="p4acc")
                        if flags['wob']:
                            nc.tensor.matmul(ps[:, :], ones_row[:, :],
                                             wob_t[:, n_ * 512:(n_ + 1) * 512],
                                             start=True, stop=False)
                        for k in range(16):
                            nc.tensor.matmul(ps[:, :],
                                             lt[k][:, m * 128:(m + 1) * 128],
                                             rh[k][:, :],
                                             start=(k == 0 and not flags['wob']),
                                             stop=(k == 15))
                        ob = p4o.tile([128, 512], F32, tag="ob")
                        nc.scalar.activation(out=ob[:, :], in_=ps[:, :], func=AF.Copy)
                        nc.sync.dma_start(
                            out=out_c[m * 128:(m + 1) * 128,
                                      n_ * 512:(n_ + 1) * 512],
                            in_=ob[:, :])

    nc.finalize()
    return nc


_ROPE_PERM = np.concatenate([np.arange(0, ROPE, 2), np.arange(1, ROPE, 2)])

_CACHE = {}


def _prep_inputs(inputs):
    """Host-side slicing/permutation -> (schedule key data, per-core in_maps)."""
    x = np.ascontiguousarray(np.asarray(inputs['x'], np.float32).reshape(N, DIM))
    mask = np.asarray(inputs['mask'])
    cls, blocks = _classify_mask(mask)

    cos_t, sin_t = _rope_tables()            # [32, S]
    trigq1 = np.concatenate([cos_t, sin_t, cos_t, sin_t], 0)   # [128, S]
    trigq2 = np.concatenate([sin_t, cos_t, sin_t, cos_t], 0)
    sgn = np.concatenate([-np.ones(32), np.ones(32), -np.ones(32), np.ones(32)]
                         ).astype(np.float32)[:, None]

    wq_a = np.asarray(inputs['wq_a_w'], np.float32)            # [QLR, DIM]
    wkv_a = np.asarray(inputs['wkv_a_w'], np.float32)          # [KVLR+ROPE, DIM]
    wkv_a_p = np.concatenate([wkv_a[:KVLR], wkv_a[KVLR:][_ROPE_PERM]], 0)
    bkva = np.asarray(inputs['wkv_a_b'], np.float32)
    bkva_p = np.concatenate([bkva[:KVLR], bkva[KVLR:][_ROPE_PERM]], 0)

    wq_b = np.asarray(inputs['wq_b_w'], np.float32).reshape(H, QKD, QLR)
    bq_b = np.asarray(inputs['wq_b_b'], np.float32).reshape(H, QKD)
    wkv_b = np.asarray(inputs['wkv_b_w'], np.float32).reshape(H, NOPE + VD, KVLR)
    bkv_b = np.asarray(inputs['wkv_b_b'], np.float32).reshape(H, NOPE + VD)
    wo = np.asarray(inputs['wo_w'], np.float32)                # [DIM, H*VD]

    shared = {
        'wqaT': np.ascontiguousarray(wq_a.T),
        'bqa': np.asarray(inputs['wq_a_b'], np.float32),
        'wkvaT': np.ascontiguousarray(wkv_a_p.T),
        'bkva': bkva_p,
        'qnw': np.asarray(inputs['q_norm_w'], np.float32),
        'kvnw': np.asarray(inputs['kv_norm_w'], np.float32),
        'trigq1': np.ascontiguousarray(trigq1),
        'trigq2': np.ascontiguousarray(trigq2),
        'sgn': sgn,
        'woT': np.ascontiguousarray(wo.T),
        'wob': np.asarray(inputs['wo_b'], np.float32)[None, :],
        'maskblk': blocks if len(blocks) else np.zeros((1, 128, 512), np.float32),
    }

    in_maps = []
    for c in range(NC):
        h0, h1 = 2 * c, 2 * c + 1
        # q_b rows: h0 nope, h1 nope, [h0 rope-e, h0 rope-o, h1 rope-e, h1 rope-o]
        wqb_c = np.concatenate([
            wq_b[h0, :NOPE], wq_b[h1, :NOPE],
            wq_b[h0, NOPE:][_ROPE_PERM], wq_b[h1, NOPE:][_ROPE_PERM]], 0)
        bqb_c = np.concatenate([
            bq_b[h0, :NOPE], bq_b[h1, :NOPE],
            bq_b[h0, NOPE:][_ROPE_PERM], bq_b[h1, NOPE:][_ROPE_PERM]], 0)
        wkb_c = np.concatenate([wkv_b[h0, :NOPE], wkv_b[h1, :NOPE]], 0)
        bkb_c = np.concatenate([bkv_b[h0, :NOPE], bkv_b[h1, :NOPE]], 0)
        wvb_c = np.concatenate([wkv_b[h0, NOPE:], wkv_b[h1, NOPE:]], 0)
        bvb_c = np.concatenate([bkv_b[h0, NOPE:], bkv_b[h1, NOPE:]], 0)
        pos = (c % (S // R)) * R
        cos_c, sin_c = cos_t[:, pos:pos + R], sin_t[:, pos:pos + R]
        m = dict(shared)
        m.update({
            'x_c': x[c * R:(c + 1) * R],
            'wqbT': np.ascontiguousarray(wqb_c.T),
            'bqb': bqb_c,
            'wkbT': np.ascontiguousarray(wkb_c.T),
            'bkb': bkb_c,
            'wvbT': np.ascontiguousarray(wvb_c.T),
            'bvb': bvb_c[None, :],
            'trig1c': np.ascontiguousarray(np.concatenate([cos_c, sin_c], 0)),
            'trig2c': np.ascontiguousarray(np.concatenate([sin_c, cos_c], 0)),
        })
        in_maps.append(m)
    return cls, in_maps


class _Runner:
    """Compile once, execute many times on the 8 axon-tunneled NeuronCores."""

    def __init__(self, nc):
        import jax
        from jax.experimental.shard_map import shard_map
        from jax.sharding import Mesh, PartitionSpec
        from concourse import bass2jax, mybir as _mybir
        bass2jax.install_neuronx_cc_hook()
        self.jax = jax
        in_names, out_names, out_avals, zero_outs = [], [], [], []
        partition_name = (nc.partition_id_tensor.name
                          if nc.partition_id_tensor else None)
        for alloc in nc.m.functions[0].allocations:
            if not isinstance(alloc, _mybir.MemoryLocationSet):
                continue
            name = alloc.memorylocations[0].name
            if alloc.kind == "ExternalInput":
                if name != partition_name:
                    in_names.append(name)
            elif alloc.kind == "ExternalOutput":
                shape = tuple(alloc.tensor_shape)
                dtype = _mybir.dt.np(alloc.dtype)
                out_names.append(name)
                out_avals.append(jax.core.ShapedArray(shape, dtype))
                zero_outs.append(np.zeros(shape, dtype))
        self.n_params = len(in_names)
        self.in_names = list(in_names)
        self.out_names = out_names
        self.out_avals = out_avals
        self.zero_outs = zero_outs
        all_in = in_names + out_names
        if partition_name is not None:
            all_in.append(partition_name)

        def _body(*args):
            operands = list(args)
            if partition_name is not None:
                operands.append(bass2jax.partition_id_tensor())
            outs = bass2jax._bass_exec_p.bind(
                *operands,
                out_avals=tuple(out_avals),
                in_names=tuple(all_in),
                out_names=tuple(out_names),
                lowering_input_output_aliases=(),
                sim_require_finite=True,
                sim_require_nnan=True,
                nc=nc)
            return tuple(outs)

        devices = jax.devices()[:NC]
        self.mesh = Mesh(np.asarray(devices), ("core",))
        n_out = len(out_names)
        in_specs = (PartitionSpec("core"),) * (self.n_params + n_out)
        out_specs = (PartitionSpec("core"),) * n_out
        donate = tuple(range(self.n_params, self.n_params + n_out))
        self.fn = jax.jit(
            shard_map(_body, mesh=self.mesh, in_specs=in_specs,
                      out_specs=out_specs, check_rep=False),
            donate_argnums=donate, keep_unused=True)

    def concat_inputs(self, in_maps):
        return [np.concatenate([np.asarray(in_maps[c][nm])
                                for c in range(NC)], axis=0)
                for nm in self.in_names]

    def zeros(self):
        return [np.zeros((NC * z.shape[0], *z.shape[1:]), z.dtype)
                for z in self.zero_outs]

    def __call__(self, concat_in, concat_zeros):
        out = self.fn(*concat_in, *concat_zeros)
        return out

    def run(self, in_maps):
        outs = self(self.concat_inputs(in_maps), self.zeros())
        res = []
        for c in range(NC):
            res.append({nm: np.asarray(outs[i]).reshape(NC, *self.out_avals[i].shape)[c]
                        for i, nm in enumerate(self.out_names)})
        return res


def _get_exec(cls, nmask, flags):
    key = (tuple(tuple(r) for r in cls), nmask,
           tuple(flags['ba']), flags['bqb'], flags['bkb'], flags['bvb'],
           flags['wob'])
    if key not in _CACHE:
        nc = _build(cls, nmask, flags)
        _CACHE[key] = _Runner(nc)
    return _CACHE[key]


def kernel(**inputs):
    cls, in_maps = _prep_inputs(inputs)
    nmask = max(len(in_maps[0]['maskblk']), 1)
    flags = {
        'ba': (bool(np.any(inputs['wq_a_b'])), bool(np.any(inputs['wkv_a_b']))),
        'bqb': bool(np.any(inputs['wq_b_b'])),
        'bkb': bool(np.any(np.asarray(inputs['wkv_b_b']).reshape(H, NOPE + VD)[:, :NOPE])),
        'bvb': bool(np.any(np.asarray(inputs['wkv_b_b']).reshape(H, NOPE + VD)[:, NOPE:])),
        'wob': bool(np.any(inputs['wo_b'])),
    }
    runner = _get_exec(cls, nmask, flags)
    results = runner.run(in_maps)
    out = np.concatenate([results[c]["out"] for c in range(NC)], 0)
    return out.reshape(B, S, DIM)
